# revision 3
# baseline (speedup 1.0000x reference)
"""Trainium2 Bass kernel for GRU + ragged unpad + L2 normalize — block-Picard.

Key idea: instead of a serial scan with a full [1024x3072] weight sweep per
timestep (192 tiny matmuls/step), process blocks of S timesteps with Picard
fixed-point iteration: freeze the recurrent matvec input at the previous
iterate's trajectory, compute all S steps' gate pre-activations in one wide
GEMM (moving dim = 2 seqs x S steps), then solve the remaining DIAGONAL
linear recurrence h_t = z_t*h_{t-1} + (1-z_t)*n_t exactly with the DVE's
tensor_tensor_scan instruction. Contraction factor ~0.23/iter, so M=6
iterations reach ~1e-4 — far below the fp8/bf16 noise floor.

Per core: 2 sequences (data-parallel over batch across 8 cores).
Everything fused in one pass per block: x DMA -> xg GEMM (biases folded,
bf16) -> M Picard iterations (fp8 weights, identity-matmul folds xg into
the psum for r/z gates) -> L2 normalize -> yout DMA.
"""

import numpy as np
import ml_dtypes

B, T, D = 16, 2048, 1024
G3 = 3 * D
NCORES = 8
BPC = B // NCORES    # 2 sequences per core
KC = D // 128        # 8 contraction chunks
MC = G3 // 128       # 24 gate chunks
HC = D // 128        # 8 hidden chunks
S = 230              # Picard block size (2S <= 512 fits one PSUM bank)
M_ITERS = 4
EPS = 1e-12

_cache = {}


def _build(nb: int, m_iters: int = M_ITERS, s: int = S, repeat: int = 1):
    """repeat>1 wraps the whole body in a For_i hardware loop that re-runs
    the identical computation; used only for differential timing."""
    import concourse.mybir as mybir
    import concourse.tile as tile
    from concourse import bacc
    from contextlib import nullcontext

    f32 = mybir.dt.float32
    bf16 = mybir.dt.bfloat16
    fp8 = mybir.dt.float8e4
    AF = mybir.ActivationFunctionType
    ALU = mybir.AluOpType

    tc = nb * s
    nc = bacc.Bacc("TRN2", enable_partition_id=False)

    xT = nc.dram_tensor("xT", [KC, 128, BPC, tc], bf16, kind="ExternalInput")
    wihT = nc.dram_tensor("wihT", [KC, 128, G3], bf16, kind="ExternalInput")
    whhT = nc.dram_tensor("whhT", [KC, 128, G3], fp8, kind="ExternalInput")
    biasA_d = nc.dram_tensor("biasA", [128, MC], f32, kind="ExternalInput")
    bhn_d = nc.dram_tensor("bhn", [128, HC], f32, kind="ExternalInput")
    yout = nc.dram_tensor("yout", [HC, 128, BPC, tc], f32, kind="ExternalOutput")

    with tile.TileContext(nc) as tc_:
        with (
            tc_.tile_pool(name="persist", bufs=1) as pp,
            tc_.tile_pool(name="xb", bufs=2) as pxb,
            tc_.tile_pool(name="ps", bufs=4, space="PSUM") as pps,
            tc_.tile_pool(name="psn", bufs=2, space="PSUM") as ppsn,
            tc_.tile_pool(name="pcs", bufs=1, space="PSUM") as pcs,
            tc_.tile_pool(name="pcb", bufs=1, space="PSUM") as pcb,
            tc_.tile_pool(name="sq", bufs=2) as psq,
            tc_.tile_pool(name="yo", bufs=2) as pyo,
        ):
            wih_sb = pp.tile([128, KC, G3], bf16, tag="wih")
            whh_sb = pp.tile([128, KC, G3], fp8, tag="whh")
            biasA = pp.tile([128, MC], f32, tag="biasA")
            bhn = pp.tile([128, HC], f32, tag="bhn")
            # Picard trajectory ping-pong: slot 0 = block-entry h, 1..S = steps
            HA = pp.tile([128, KC, BPC, s + 1], bf16, tag="HA")
            HB = pp.tile([128, KC, BPC, s + 1], bf16, tag="HB")
            hent = pp.tile([128, KC, BPC, 1], f32, tag="hent")
            # r/z chunks kept f32 (added into PSUM by DVE each iteration);
            # n chunks bf16 (added to the bf16 n-path on DVE)
            xgrz = pp.tile([128, 2 * HC, BPC, s], f32, tag="xgrz")
            xgn = pp.tile([128, HC, BPC, s], bf16, tag="xgn")
            zt = pp.tile([128, HC, BPC, s], bf16, tag="zt")
            rt = pp.tile([128, HC, BPC, s], bf16, tag="rt")
            nt = pp.tile([128, HC, BPC, s], bf16, tag="nt")
            nt2 = pp.tile([128, HC, BPC, s], bf16, tag="nt2")
            zs = pp.tile([128, s], bf16, tag="zs")
            ones_k = pp.tile([128, 1], bf16, tag="ones_k")
            ones_m = pp.tile([1, 128], bf16, tag="ones_m")
            rsb = pp.tile([1, s], bf16, tag="rsb")
            nrm = pp.tile([1, s], f32, tag="nrm")
            rsf = pp.tile([1, s], f32, tag="rsf")

            for k in range(KC):
                nc.sync.dma_start(out=wih_sb[:, k, :], in_=wihT[k, :, :])
                nc.sync.dma_start(out=whh_sb[:, k, :], in_=whhT[k, :, :])
            nc.sync.dma_start(out=biasA, in_=biasA_d[:, :])
            nc.sync.dma_start(out=bhn, in_=bhn_d[:, :])
            nc.vector.memset(zs, 0.0)
            nc.vector.memset(ones_k, 1.0)
            nc.vector.memset(ones_m, 1.0)
            nc.vector.memset(HA[:, :, :, 0:1], 0.0)
            nc.vector.memset(HB[:, :, :, 0:1], 0.0)
            nc.vector.memset(hent, 0.0)

            bufs = [HA, HB]
            rep_ctx = (
                tc_.For_i(
                    0, repeat, 1,
                    hint_engines=(
                        mybir.EngineType.PE,
                        mybir.EngineType.DVE,
                        mybir.EngineType.Activation,
                    ),
                )
                if repeat > 1 else nullcontext()
            )
            with rep_ctx:
                _body(nc, tile, mybir, nb, m_iters, s, bufs, locals())

    nc.compile()
    return nc


def _never():  # placeholder to keep indentation sane
    pass


def _body(nc, tile, mybir, nb, m_iters, s, bufs, env):
    f32 = mybir.dt.float32
    bf16 = mybir.dt.bfloat16
    AF = mybir.ActivationFunctionType
    ALU = mybir.AluOpType
    (HA, HB, hent, xgrz, xgn, zt, rt, nt, nt2, zs, ones_k, ones_m, rsb, nrm,
     rsf, wih_sb, whh_sb, biasA, bhn, xT, yout, pxb, pps, ppsn, pcs, pcb,
     psq, pyo) = (
        env["HA"], env["HB"], env["hent"], env["xgrz"], env["xgn"],
        env["zt"], env["rt"],
        env["nt"], env["nt2"], env["zs"], env["ones_k"], env["ones_m"],
        env["rsb"], env["nrm"], env["rsf"], env["wih_sb"], env["whh_sb"],
        env["biasA"], env["bhn"], env["xT"], env["yout"],
        env["pxb"], env["pps"], env["ppsn"], env["pcs"], env["pcb"],
        env["psq"], env["pyo"],
    )
    for blk in range(nb):
                t0 = blk * s
                if blk > 0:
                    # entry h := final h of previous block (in HY)
                    HY = bufs[m_iters % 2]
                    nc.vector.tensor_copy(HA[:, :, :, 0:1], HY[:, :, :, s:s+1])
                    nc.vector.tensor_copy(HB[:, :, :, 0:1], HY[:, :, :, s:s+1])
                    nc.vector.tensor_copy(hent, HY[:, :, :, s:s+1])
                # iteration-0 trajectory: constant = entry h
                for j in range(KC):
                    for b in range(BPC):
                        nc.vector.tensor_scalar_add(
                            HA[:, j, b, 1:s+1], zs, hent[:, j, b, :]
                        )

                xb = pxb.tile([128, KC, BPC, s], bf16, tag="xb")
                for k in range(KC):
                    nc.sync.dma_start(
                        out=xb[:, k, :, :], in_=xT[k, :, :, t0:t0+s]
                    )
                # xg = x @ w_ih.T + biasA (bih, + bhh folded for r/z)
                for m in range(MC):
                    ps = pps.tile([128, BPC, s], f32, tag="ps")
                    for k in range(KC):
                        nc.tensor.matmul(
                            ps, wih_sb[:, k, m*128:(m+1)*128], xb[:, k, :, :],
                            start=(k == 0), stop=(k == KC - 1),
                        )
                    dst = (xgrz[:, m, :, :] if m < 2 * HC
                           else xgn[:, m - 2 * HC, :, :])
                    nc.scalar.activation(
                        dst, ps, AF.Identity, bias=biasA[:, m:m+1],
                    )

                for it in range(m_iters):
                    HR = bufs[it % 2]
                    HW = bufs[1 - it % 2]
                    for j in range(HC):
                        # r gate (m-chunk j); xg added into psum by DVE
                        pr = pps.tile([128, BPC, s], f32, tag="ps")
                        for k in range(KC):
                            nc.tensor.matmul(
                                pr, whh_sb[:, k, j*128:(j+1)*128],
                                HR[:, k, :, 0:s],
                                start=(k == 0), stop=(k == KC - 1),
                            )
                        nc.vector.tensor_add(pr, pr, xgrz[:, j, :, :])
                        nc.scalar.activation(rt[:, j, :, :], pr, AF.Sigmoid)
                        # z gate (m-chunk 8+j)
                        pz = pps.tile([128, BPC, s], f32, tag="ps")
                        for k in range(KC):
                            nc.tensor.matmul(
                                pz, whh_sb[:, k, D + j*128:D + (j+1)*128],
                                HR[:, k, :, 0:s],
                                start=(k == 0), stop=(k == KC - 1),
                            )
                        nc.vector.tensor_add(pz, pz, xgrz[:, HC + j, :, :])
                        nc.scalar.activation(zt[:, j, :, :], pz, AF.Sigmoid)
                        # n pre-activation (m-chunk 16+j), no xg fold
                        pn = ppsn.tile([128, BPC, s], f32, tag="psn")
                        for k in range(KC):
                            nc.tensor.matmul(
                                pn, whh_sb[:, k, 2*D + j*128:2*D + (j+1)*128],
                                HR[:, k, :, 0:s],
                                start=(k == 0), stop=(k == KC - 1),
                            )
                        # t = (g_n + bhn_j) * r  (PSUM-in0 stt wedges the HW,
                        # so drain via ACT identity+bias first)
                        nc.scalar.activation(
                            nt[:, j, :, :], pn, AF.Identity,
                            bias=bhn[:, j:j+1],
                        )
                        nc.vector.tensor_mul(
                            nt[:, j, :, :], nt[:, j, :, :], rt[:, j, :, :]
                        )
                        nc.vector.tensor_add(
                            nt[:, j, :, :], nt[:, j, :, :],
                            xgn[:, j, :, :],
                        )
                        nc.scalar.activation(
                            nt2[:, j, :, :], nt[:, j, :, :], AF.Tanh
                        )
                        # nb = (z - 1) * n   (so h = z*h - nb)
                        nc.vector.scalar_tensor_tensor(
                            nt[:, j, :, :], zt[:, j, :, :], -1.0,
                            nt2[:, j, :, :], ALU.add, ALU.mult,
                        )
                        for b in range(BPC):
                            nc.vector.tensor_tensor_scan(
                                HW[:, j, b, 1:s+1], zt[:, j, b, :],
                                nt[:, j, b, :], hent[:, j, b, :],
                                ALU.mult, ALU.subtract,
                            )

                # normalize + emit this block from the final buffer HY
                HY = bufs[m_iters % 2]
                for b in range(BPC):
                    pss = pcs.tile([1, s], f32, tag="pss")
                    hks = []
                    for k in range(KC):
                        hk = psq.tile([128, s], f32, tag=f"hk{k}")
                        nc.vector.tensor_copy(hk, HY[:, k, b, 1:s+1])
                        hks.append(hk)
                        sq = psq.tile([128, s], bf16, tag="sq")
                        nc.vector.tensor_mul(
                            sq, HY[:, k, b, 1:s+1], HY[:, k, b, 1:s+1]
                        )
                        nc.tensor.matmul(
                            pss, ones_k, sq,
                            start=(k == 0), stop=(k == KC - 1),
                        )
                    nc.scalar.activation(nrm, pss, AF.Sqrt)
                    nc.vector.tensor_scalar_max(nrm, nrm, EPS)
                    nc.vector.reciprocal(rsf, nrm)
                    nc.vector.tensor_copy(rsb, rsf)
                    pbc = pcb.tile([128, s], f32, tag="pbc")
                    nc.tensor.matmul(pbc, ones_m, rsb, start=True, stop=True)
                    for k in range(KC):
                        yo = pyo.tile([128, s], f32, tag="yo")
                        nc.vector.tensor_mul(yo, hks[k], pbc)
                        nc.sync.dma_start(
                            out=yout[k, :, b, t0:t0+s], in_=yo
                        )


def _build_noop(nb: int):
    """Same I/O signature as _build but a trivial body — used by test2.py to
    subtract dispatch/transfer overhead from wall-clock timing."""
    import concourse.mybir as mybir
    import concourse.tile as tile
    from concourse import bacc

    f32 = mybir.dt.float32
    bf16 = mybir.dt.bfloat16
    fp8 = mybir.dt.float8e4
    tc = nb * S
    nc = bacc.Bacc("TRN2", enable_partition_id=False)
    nc.dram_tensor("xT", [KC, 128, BPC, tc], bf16, kind="ExternalInput")
    nc.dram_tensor("wihT", [KC, 128, G3], bf16, kind="ExternalInput")
    nc.dram_tensor("whhT", [KC, 128, G3], fp8, kind="ExternalInput")
    biasA = nc.dram_tensor("biasA", [128, MC], f32, kind="ExternalInput")
    nc.dram_tensor("bhn", [128, HC], f32, kind="ExternalInput")
    yout = nc.dram_tensor("yout", [HC, 128, BPC, tc], f32,
                          kind="ExternalOutput")
    with tile.TileContext(nc) as tc_:
        with tc_.tile_pool(name="p", bufs=1) as p:
            t = p.tile([128, MC], f32, tag="t")
            nc.sync.dma_start(out=t, in_=biasA[:, :])
            nc.sync.dma_start(out=yout[0, :, 0, :MC], in_=t)
    nc.compile()
    return nc


def _prep_inputs(x, w_ih, w_hh, b_ih, b_hh, tc):
    """Host-side layout prep (not timed): transposes + dtype casts."""
    bf = ml_dtypes.bfloat16
    f8 = ml_dtypes.float8_e4m3
    x = np.asarray(x, np.float32)
    wihT = np.ascontiguousarray(np.asarray(w_ih, np.float32).T).astype(bf)
    whhT = np.ascontiguousarray(np.asarray(w_hh, np.float32).T).astype(f8)
    wihT = wihT.reshape(KC, 128, G3)
    whhT = whhT.reshape(KC, 128, G3)
    biasA = np.asarray(b_ih, np.float32).copy()
    biasA[:2*D] += np.asarray(b_hh, np.float32)[:2*D]
    biasA = np.ascontiguousarray(biasA.reshape(MC, 128).T)
    bhn = np.ascontiguousarray(
        np.asarray(b_hh, np.float32)[2*D:].reshape(HC, 128).T
    )
    in_maps = []
    for c in range(NCORES):
        xc = x[c*BPC:(c+1)*BPC, :tc]                  # [2, tc, D]
        xTc = np.ascontiguousarray(xc.transpose(2, 0, 1))  # [D, 2, tc]
        xTc = xTc.reshape(KC, 128, BPC, tc).astype(bf)
        in_maps.append({
            "xT": xTc, "wihT": wihT, "whhT": whhT,
            "biasA": biasA, "bhn": bhn,
        })
    return in_maps


def _assemble(results, lengths, tc):
    """Per-core yout [HC,128,BPC,tc] f32 -> flat [sum(lengths), D]."""
    lengths = np.asarray(lengths).astype(np.int64)
    parts = []
    for c in range(NCORES):
        yo = np.asarray(results[c]["yout"], np.float32)
        yo = yo.reshape(D, BPC, tc).transpose(1, 2, 0)  # [2, tc, D]
        for b in range(BPC):
            parts.append(yo[b, :lengths[c*BPC + b]])
    return np.concatenate(parts, axis=0)


def kernel(x, lengths, w_ih, w_hh, b_ih, b_hh):
    from concourse import bass_utils

    lengths_np = np.asarray(lengths).astype(np.int64)
    max_len = int(lengths_np.max())
    nb = -(-max_len // S)
    tc = nb * S
    key = (nb, M_ITERS, S)
    if key not in _cache:
        _cache[key] = _build(nb)
    nc = _cache[key]

    in_maps = _prep_inputs(x, w_ih, w_hh, b_ih, b_hh, tc)
    res = bass_utils.run_bass_kernel_spmd(nc, in_maps, list(range(NCORES)))
    return _assemble(res.results, lengths_np, tc)


if __name__ == "__main__":
    import reference

    inputs = reference.setup_inputs()
    out = kernel(**{k: np.asarray(v) for k, v in inputs.items()})
    exp = np.asarray(reference.reference(**inputs))
    err = np.abs(out - exp).max()
    rel = np.linalg.norm(out - exp) / np.linalg.norm(exp)
    print("absmax:", err, "rel:", rel)
